# revision 1
# baseline (speedup 1.0000x reference)
"""Trainium2 Bass kernel for nn_AutoregressiveConvLSTM.

Data-parallel over batch: 32 images -> 8 cores x 4 images.

Layout per core: every 2D field (x channel, zi channel, h, c, gates) is stored
as (partition = H row 0..127, free = img*130 + 1 + w) with zero pad columns at
w offsets 0 and 129 of each image so that the 3 horizontal conv taps are plain
free-dim offset reads (dx in 0..2).

3x3 SAME convs run on the TensorEngine as banded matmuls: out = B.T @ rhs with
B[h, h'] = W[h-h'+1, dx, ci, co] a tridiagonal 128x128 "band" (vertical taps),
one matmul per (dx, ci) accumulating in PSUM; rhs is the plane with the
free-dim offset dx.

Recurrence avoids ACT table switches by using only tanh + exp
(exp_and_others set):  sigmoid(v) = 0.5*(tanh(v/2)+1).  h is stored doubled
(h2 = 2h = (tanh(o/2)+1)*tanh(c)) and the 0.5 is folded into the Whh/Wout
bands.

log prob: z = ((mu+b0) - x) * exp(-ls-b1); per-image Sum z^2 via ACT Square
with accum_out; Sum ls via DVE tensor_reduce; final cross-partition reduction
via a ones-vector matmul.
"""

import sys
import numpy as np

for _p in ("/opt/trn_rl_repo", "/root/.axon_site/_ro/trn_rl_repo"):
    if _p not in sys.path:
        sys.path.insert(0, _p)

import concourse.bacc as bacc
import concourse.mybir as mybir
from concourse import bass, tile
from concourse.bass_utils import run_bass_kernel_spmd

F32 = mybir.dt.float32
F32R = mybir.dt.float32r
AF = mybir.ActivationFunctionType
ALU = mybir.AluOpType

B, C, H, W = 32, 16, 128, 128
NCORES = 8
BL = B // NCORES          # images per core = 4
WP = W + 2                # padded row width = 130
LOG2PI = 1.8378770664093453

# band tensor indexing
N_IN = 3                        # conv_in: dx
N_IH = 8 * 3                    # conv_ih: co, dx
N_HH = 8 * 2 * 3                # conv_hh: co, ci, dx
N_OUT = 2 * 2 * 3               # conv_out: co, ci, dx
NBANDS = N_IN + N_IH + N_HH + N_OUT   # 87


def _band(w_col):
    """Build the 128x128 tridiagonal lhsT for one (ky tap column) of a 3-tap
    vertical conv: lhsT[h, h'] = w_col[h - h' + 1] for |h-h'| <= 1."""
    Bm = np.zeros((H, H), np.float32)
    idx = np.arange(H)
    for ky in range(3):
        hh = idx + ky - 1          # input row feeding output row idx
        m = (hh >= 0) & (hh < H)
        Bm[hh[m], idx[m]] = w_col[ky]
    return Bm


def _build_bands(Win, Wih, Whh, Wout):
    """All band matrices as one (87,128,128) array (lhsT layout)."""
    bands = np.zeros((NBANDS, H, H), np.float32)
    k = 0
    for dx in range(3):                        # conv_in (1->1)
        bands[k] = _band(Win[:, dx, 0, 0]); k += 1
    for co in range(8):                        # conv_ih (1->8)
        for dx in range(3):
            bands[k] = _band(Wih[:, dx, 0, co]); k += 1
    for co in range(8):                        # conv_hh (2->8), x0.5 (h2)
        for ci in range(2):
            for dx in range(3):
                bands[k] = _band(0.5 * Whh[:, dx, ci, co]); k += 1
    Wout_y = Wout[:, :, :2, :]                 # cond features are zero
    for co in range(2):                        # conv_out (2->2), x0.5 (h2)
        for ci in range(2):
            for dx in range(3):
                bands[k] = _band(0.5 * Wout_y[:, dx, ci, co]); k += 1
    assert k == NBANDS
    return bands


_CACHED = None


def _build_program(nsteps=None, skip_rec=False):
    import os
    if nsteps is None:
        nsteps = int(os.environ.get("KERNEL_T", C - 1))
    nc = bacc.Bacc(None, target_bir_lowering=False)

    xp_d = nc.dram_tensor("xp", [H, C * BL * WP], F32, kind="ExternalInput")
    bands_d = nc.dram_tensor("bands", [H, NBANDS * H], F32R, kind="ExternalInput")
    cols_d = nc.dram_tensor("cols", [H, 16], F32, kind="ExternalInput")
    out_d = nc.dram_tensor("out", [BL, 1], F32, kind="ExternalOutput")

    T = C - 1  # 15 recurrence steps
    TR = nsteps

    with tile.TileContext(nc) as tc:
        with (
            tc.tile_pool(name="const", bufs=1) as cpool,
            tc.tile_pool(name="state", bufs=1) as spool,
            tc.tile_pool(name="work", bufs=2) as wpool,
            tc.tile_pool(name="once", bufs=1) as opool,
            tc.tile_pool(name="psum", bufs=4, space=bass.MemorySpace.PSUM) as ppool,
        ):
            xall = cpool.tile([H, C, BL, WP], F32, tag="xall")
            bandsb = cpool.tile([H, NBANDS, H], F32R, tag="bands")
            cols = cpool.tile([H, 16], F32, tag="cols")
            ziall = cpool.tile([H, T, BL, WP], F32R, tag="ziall")
            ones = cpool.tile([H, 1], F32, tag="ones")

            hpair = spool.tile([H, 2, BL, WP], F32R, tag="hpair")
            cst = spool.tile([H, 2, BL, W], F32, tag="cst")
            sqcols = spool.tile([H, BL, C], F32, tag="sqcols")
            lscols = spool.tile([H, BL, C], F32, tag="lscols")

            # ---- load inputs ----
            nc.sync.dma_start(xall[:], xp_d[:])
            nc.sync.dma_start(bandsb[:], bands_d[:])
            nc.sync.dma_start(cols[:], cols_d[:])

            nc.gpsimd.memset(ziall[:].bitcast(F32), 0.0)
            nc.gpsimd.memset(hpair[:].bitcast(F32), 0.0)
            nc.gpsimd.memset(cst[:], 0.0)
            nc.gpsimd.memset(sqcols[:], 0.0)
            nc.gpsimd.memset(lscols[:], 0.0)
            nc.gpsimd.memset(ones[:], 1.0)

            def band(i):
                return bandsb[:, i, :]

            # ---- precompute zi_t = conv_in(x_t) + b_in for t in 0..14 ----
            for t in range(T):
                xr = wpool.tile([H, BL, WP], F32R, tag="xr")
                nc.vector.tensor_copy(xr[:], xall[:, t])
                zps = ppool.tile([H, BL, W], F32, tag="ps")
                for dx in range(3):
                    nc.tensor.matmul(
                        zps[:], band(dx), xr[:, :, dx:dx + W],
                        start=(dx == 0), stop=(dx == 2),
                    )
                # zi -> SBUF padded cols, +b_in
                nc.scalar.activation(
                    ziall[:, t, :, 1:1 + W], zps[:], AF.Identity,
                    bias=cols[:, 0:1],
                )

            # ---- channel 0 logprob: z0 = (x0 - b0) * exp(-b1) ----
            # Square(scale*x + bias) with scale=e^{-b1}, bias=-b0*e^{-b1}
            zjunk = opool.tile([H, BL, W], F32, tag="zjunk")
            for im in range(BL):
                nc.scalar.activation(
                    zjunk[:, im, :], xall[:, 0, im, 1:1 + W], AF.Square,
                    scale=cols[:, 2:3], bias=cols[:, 3:4],
                    accum_out=sqcols[:, im, C - 1:C],
                )

            # ---- recurrence ----
            for t in range(0 if skip_rec else TR):
                th = []  # tanh-gate tiles: i,g,f,o
                for g in range(4):
                    gps = ppool.tile([H, 2, BL, W], F32, tag="ps")
                    for half in range(2):
                        co = g * 2 + half
                        mms = []
                        for dx in range(3):
                            mms.append((N_IN + co * 3 + dx,
                                        ziall[:, t, :, dx:dx + W]))
                        if t > 0:
                            for ci in range(2):
                                for dx in range(3):
                                    mms.append((
                                        N_IN + N_IH + (co * 2 + ci) * 3 + dx,
                                        hpair[:, ci, :, dx:dx + W]))
                        for k, (bi, rhs) in enumerate(mms):
                            nc.tensor.matmul(
                                gps[:, half], band(bi), rhs,
                                start=(k == 0), stop=(k == len(mms) - 1),
                            )
                    tg = wpool.tile([H, 2, BL, W], F32, tag=f"th{g}")
                    # i,f,o: tanh(v/2 + bias'); g: tanh(v + bias)
                    scale = 1.0 if g == 1 else 0.5
                    for half in range(2):
                        co = g * 2 + half
                        nc.scalar.activation(
                            tg[:, half], gps[:, half], AF.Tanh,
                            scale=scale, bias=cols[:, 5 + co:6 + co],
                        )
                    th.append(tg)
                ti, tgg, tf, to = th

                u1 = wpool.tile([H, 2, BL, W], F32, tag="u1")
                u2 = wpool.tile([H, 2, BL, W], F32, tag="u2")
                nc.vector.scalar_tensor_tensor(
                    u1[:], tf[:], 1.0, cst[:], ALU.add, ALU.mult)
                nc.vector.scalar_tensor_tensor(
                    u2[:], ti[:], 1.0, tgg[:], ALU.add, ALU.mult)
                s2 = opool.tile([H, 2, BL, W], F32, tag="s2")
                nc.vector.tensor_add(s2[:], u1[:], u2[:])        # s2 = 2*c'
                nc.vector.tensor_scalar_mul(cst[:], s2[:], 0.5)  # c' state
                tcn = opool.tile([H, 2, BL, W], F32, tag="tcn")
                nc.scalar.activation(tcn[:], s2[:], AF.Tanh, scale=0.5)
                # h2 = (tanh(o/2)+1)*tanh(c), written into padded h tensor
                nc.vector.scalar_tensor_tensor(
                    hpair[:, :, :, 1:1 + W], to[:], 1.0, tcn[:],
                    ALU.add, ALU.mult)

                # conv_out -> mu (co 0), ls (co 1)
                pps = ppool.tile([H, 2, BL, W], F32, tag="ps")
                for co in range(2):
                    k = 0
                    for ci in range(2):
                        for dx in range(3):
                            nc.tensor.matmul(
                                pps[:, co],
                                band(N_IN + N_IH + N_HH + (co * 2 + ci) * 3 + dx),
                                hpair[:, ci, :, dx:dx + W],
                                start=(k == 0), stop=(k == 5),
                            )
                            k += 1

                E = opool.tile([H, BL, W], F32, tag="E")
                nc.scalar.activation(
                    E[:], pps[:, 1], AF.Exp, scale=-1.0, bias=cols[:, 1:2])
                d = opool.tile([H, BL, W], F32, tag="d")
                nc.vector.scalar_tensor_tensor(
                    d[:], pps[:, 0], cols[:, 4:5], xall[:, t + 1, :, 1:1 + W],
                    ALU.add, ALU.subtract)
                z = opool.tile([H, BL, W], F32, tag="z")
                nc.vector.tensor_mul(z[:], d[:], E[:])
                zj = opool.tile([H, BL, W], F32, tag="zjunk")
                for im in range(BL):
                    nc.scalar.activation(
                        zj[:, im, :], z[:, im, :], AF.Square,
                        accum_out=sqcols[:, im, t:t + 1])
                nc.vector.tensor_reduce(
                    lscols[:, :, t:t + 1], pps[:, 1], axis=mybir.AxisListType.X,
                    op=ALU.add)

            # ---- final reduction ----
            s_sq = opool.tile([H, BL, 1], F32, tag="ssq")
            s_ls = opool.tile([H, BL, 1], F32, tag="sls")
            nc.vector.tensor_reduce(
                s_sq[:], sqcols[:], axis=mybir.AxisListType.X, op=ALU.add)
            nc.vector.tensor_reduce(
                s_ls[:], lscols[:], axis=mybir.AxisListType.X, op=ALU.add)
            comb = opool.tile([H, BL], F32, tag="comb")
            nc.vector.scalar_tensor_tensor(
                comb[:], s_sq[:, :, 0], -0.5, s_ls[:, :, 0],
                ALU.mult, ALU.subtract)
            fps = ppool.tile([BL, 1], F32, tag="ps")
            nc.tensor.matmul(fps[:], comb[:], ones[:], start=True, stop=True)
            osb = opool.tile([BL, 1], F32, tag="osb")
            nc.vector.tensor_copy(osb[:], fps[:])
            nc.sync.dma_start(out_d[:], osb[:])

    nc.compile()
    return nc


def _get_program():
    global _CACHED
    if _CACHED is None:
        _CACHED = _build_program()
    return _CACHED


def kernel(x, Win, b_in, Wih, b_ih, Whh, b_hh, Wout, b_out):
    x = np.asarray(x, np.float32)
    Win = np.asarray(Win, np.float32)
    Wih = np.asarray(Wih, np.float32)
    Whh = np.asarray(Whh, np.float32)
    Wout = np.asarray(Wout, np.float32)
    b_in = np.asarray(b_in, np.float32)
    b_ih = np.asarray(b_ih, np.float32)
    b_hh = np.asarray(b_hh, np.float32)
    b_out = np.asarray(b_out, np.float32)

    bands = _build_bands(Win, Wih, Whh, Wout)
    bands_t = np.ascontiguousarray(
        np.transpose(bands, (1, 0, 2))).reshape(H, NBANDS * H)
    bt = bands_t.view(np.uint32)
    bt += 0x1000
    bt &= np.uint32(0xFFFFE000)

    # per-partition constant columns
    cols = np.zeros((H, 16), np.float32)
    b0, b1 = float(b_out[0]), float(b_out[1])
    cols[:, 0] = float(b_in[0])
    cols[:, 1] = -b1                       # exp bias: exp(-ls - b1)
    cols[:, 2] = np.exp(-b1)               # ch0 scale
    cols[:, 3] = -b0 * np.exp(-b1)         # ch0 bias
    cols[:, 4] = b0                        # d scalar
    gb = b_ih + b_hh                       # per-co gate bias, co=[i0,i1,g0,g1,f0,f1,o0,o1]
    for co in range(8):
        g = co // 2
        if g == 1:                         # g gate: tanh(v + b)
            cols[:, 5 + co] = gb[co]
        elif g == 2:                       # f gate: tanh((v + b + 1)/2)
            cols[:, 5 + co] = 0.5 * (gb[co] + 1.0)
        else:                              # i,o: tanh((v + b)/2)
            cols[:, 5 + co] = 0.5 * gb[co]

    # padded x planes per core: (C, H, BL*WP)
    in_maps = []
    for k in range(NCORES):
        xs = x[k * BL:(k + 1) * BL]        # (BL, C, H, W)
        xpad = np.zeros((C, H, BL, WP), np.float32)
        xpad[:, :, :, 1:1 + W] = np.transpose(xs, (1, 2, 0, 3))
        in_maps.append({
            "xp": np.ascontiguousarray(
                np.transpose(xpad, (1, 0, 2, 3))).reshape(H, C * BL * WP),
            "bands": bands_t,
            "cols": cols,
        })

    nc = _get_program()
    global _last_in_maps
    _last_in_maps = in_maps
    res = run_bass_kernel_spmd(nc, in_maps, core_ids=list(range(NCORES)))

    # assemble: add host-side constants
    const = -0.5 * LOG2PI * (H * W * C) - H * W * b1   # ch0 ls sum = H*W*b1
    out = np.zeros((B,), np.float32)
    for k in range(NCORES):
        out[k * BL:(k + 1) * BL] = res.results[k]["out"].reshape(BL) + const
    return out



# revision 2
# speedup vs baseline: 1.0550x; 1.0550x over previous
"""Trainium2 Bass kernel for nn_AutoregressiveConvLSTM — v5.

v4 (fused 5x5 x->gates conv, fp8 DoubleRow, tanh+exp only) plus:

- sigma-form cell: gate tanhs stay on Act, but sigmoid values are
  materialized with tensor_scalar (t*0.5+0.5), which gets the 4x DVE mode;
  the cell is then pure tensor_tensor bf16 (2x mode) instead of
  scalar_tensor_tensor (no fast mode). h is stored plainly (no h2=2h), so
  conv_hh / conv_out bands drop their 0.5 fold.
- matmul emission per (step, pair) puts all h-independent x-tap matmuls
  first, then conv_out + conv_hh; the x-taps of the next pair cover the
  other pair's tanh/cell tail.
- z / z^2 / accz accumulation on the Pool(gpsimd) engine.
- PSUM: pfo and pco double-buffered, pig single (tanh_ig drains early), the
  final-reduce matmul shares the pco tag.
"""

import os
import sys
import numpy as np
import ml_dtypes

for _p in ("/opt/trn_rl_repo", "/root/.axon_site/_ro/trn_rl_repo"):
    if _p not in sys.path:
        sys.path.insert(0, _p)

import concourse.bacc as bacc
import concourse.mybir as mybir
from concourse import bass, tile
from concourse.bass_utils import run_bass_kernel_spmd

F32 = mybir.dt.float32
BF16 = mybir.dt.bfloat16
F8 = mybir.dt.float8e4
AF = mybir.ActivationFunctionType
ALU = mybir.AluOpType
DR = mybir.MatmulPerfMode.DoubleRow

E4NP = ml_dtypes.float8_e4m3
BFNP = ml_dtypes.bfloat16

B, C, H, W = 32, 16, 128, 128
NCORES = 8
BL = B // NCORES
WP = W + 2
WP5 = W + 4
T = C - 1
LOG2PI = 1.8378770664093453

S_G = 128.0
S_OUT = 32.0

NPAIR = 8 * 6 + 6


# x-pairs packed first so the first DMA chunk unblocks the t=0 matmuls
def PI_X(co, j):
    return co * 3 + j


def PI_H(co, dx):
    return 24 + co * 3 + dx


def PI_O(co, dx):
    return 48 + co * 3 + dx


def _band(w_col):
    Bm = np.zeros((H, H), np.float32)
    idx = np.arange(H)
    for ky in range(3):
        hh = idx + ky - 1
        m = (hh >= 0) & (hh < H)
        Bm[hh[m], idx[m]] = w_col[ky]
    return Bm


def _build_bands(Win, Wih, Whh, Wout, b_in, b_ih, b_hh):
    bp = np.zeros((NPAIR, 2, H, H), np.float32)
    B5 = np.zeros((5, 8, H, H), np.float32)
    for dx1 in range(3):
        Ain = _band(Win[:, dx1, 0, 0])
        for dx2 in range(3):
            for co in range(8):
                B5[dx1 + dx2, co] += Ain @ _band(Wih[:, dx2, 0, co])
    gb = np.asarray(b_ih, np.float32) + np.asarray(b_hh, np.float32)
    for co in range(8):
        # tanh(0.5/S_G * PSUM): sigma gates via (t+1)/2 downstream; the g
        # gate needs tanh(pre), so its bands carry 2x.
        sg = S_G * (2.0 if co in (2, 3) else 1.0)
        bp[PI_X(co, 0), 0] = B5[0, co] * sg
        bp[PI_X(co, 0), 1] = B5[1, co] * sg
        bp[PI_X(co, 1), 0] = B5[2, co] * sg
        bp[PI_X(co, 1), 1] = B5[3, co] * sg
        bp[PI_X(co, 2), 0] = B5[4, co] * sg
        gbias = (float(gb[co]) + (1.0 if co in (4, 5) else 0.0)
                 + float(b_in[0]) * float(Wih[:, :, 0, co].sum()))
        bp[PI_X(co, 2), 1] = sg * gbias / H
        for dx in range(3):
            for ci in range(2):
                bp[PI_H(co, dx), ci] = _band(Whh[:, dx, ci, co]) * sg
    for co in range(2):
        for dx in range(3):
            for ci in range(2):
                bp[PI_O(co, dx), ci] = _band(Wout[:, dx, ci, co]) * S_OUT
    return bp


_CACHED = {}

IG_CO = [0, 1, 2, 3]     # i0,i1,g0,g1
FO_CO = [4, 5, 6, 7]     # f0,f1,o0,o1


def _build_program(b_in, b_ih, b_hh, b_out, nsteps=None):
    if nsteps is None:
        nsteps = int(os.environ.get("KERNEL_T", T))
    b1 = float(b_out[1])
    nc = bacc.Bacc(None, target_bir_lowering=False)

    # xq planes: 0 = ones (device memset), 1..15 = x channels 0..14
    xq_d = nc.dram_tensor("xq", [H, (C - 1) * BL * WP5], F8,
                          kind="ExternalInput")
    xs_d = nc.dram_tensor("xs", [H, C * BL * W], BF16, kind="ExternalInput")
    bands_d = nc.dram_tensor("bands", [H, NPAIR * 2 * H], F8,
                             kind="ExternalInput")
    out_d = nc.dram_tensor("out", [BL, 1], F32, kind="ExternalOutput")

    XPITCH = C * BL * WP5
    PL5 = BL * WP5

    with tile.TileContext(nc) as tc:
        with (
            tc.tile_pool(name="const", bufs=1) as cpool,
            tc.tile_pool(name="state", bufs=1) as spool,
            tc.tile_pool(name="work", bufs=2) as wpool,
            tc.tile_pool(name="psum", bufs=1, space=bass.MemorySpace.PSUM) as p1,
            tc.tile_pool(name="psum2", bufs=2, space=bass.MemorySpace.PSUM) as p2,
        ):
            xq = cpool.tile([H, C, BL, WP5], F8, tag="xq")
            xs = cpool.tile([H, C, BL, W], BF16, tag="xs")
            bandsb = cpool.tile([H, NPAIR, 2, H], F8, tag="bands")
            ones = cpool.tile([H, 1], F32, tag="ones")
            ebias = cpool.tile([H, 1], F32, tag="ebias")

            hpair = spool.tile([H, 2, BL, WP], F8, tag="hpair")
            cst = spool.tile([H, 2, BL, W], BF16, tag="cst")
            accz = spool.tile([H, BL, W], F32, tag="accz")
            accls = spool.tile([H, BL, W], F32, tag="accls")

            # ones plane at index 0 (never DMA'd, so no WAW)
            nc.gpsimd.memset(xq[:, 0], 1.0)
            # chunked loads: x-band pairs + first x planes unblock t=0
            nc.sync.dma_start(bandsb[:, 0:24], bands_d[:, 0:24 * 2 * H])
            nc.sync.dma_start(xq[:, 1:3], xq_d[:, 0:2 * BL * WP5])
            nc.sync.dma_start(bandsb[:, 24:], bands_d[:, 24 * 2 * H:])
            nc.sync.dma_start(xq[:, 3:C], xq_d[:, 2 * BL * WP5:])
            nc.sync.dma_start(xs[:], xs_d[:])
            nc.gpsimd.memset(hpair[:], 0.0)
            nc.gpsimd.memset(cst[:], 0.0)
            nc.gpsimd.memset(accls[:], 0.0)
            nc.gpsimd.memset(ones[:], 1.0)
            nc.gpsimd.memset(ebias[:], -(b1 + float(np.log(S_OUT))))

            def band(pair):
                return bandsb[:, pair]

            def rhs_x(t, P, j):
                # x channel t lives at plane t+1; j=2's k-partner is the
                # ones plane at 0 (negative stride keeps the AP's read
                # bounding box to planes 0..t+1, so early steps don't wait
                # on the later DMA chunks)
                off = (t + 1) * PL5 + P * 2 * WP5 + 2 * j
                ks = 1 if j < 2 else -((t + 1) * PL5 + 4)
                return bass.AP(xq[:].tensor, off,
                               [[XPITCH, H], [ks, 2], [WP5, 2], [1, W]])

            def rhs_h(dx, P):
                off = dx + P * 2 * WP
                return bass.AP(hpair[:].tensor, off,
                               [[2 * BL * WP, H], [BL * WP, 2], [WP, 2], [1, W]])

            def x_mms(region, co, t, last_stop):
                for j in range(3):
                    nc.tensor.matmul(region, band(PI_X(co, j)),
                                     rhs_x(t, P_cur[0], j), start=(j == 0),
                                     stop=(last_stop and j == 2), perf_mode=DR)

            def h_mms(region, co):
                for dx in range(3):
                    nc.tensor.matmul(region, band(PI_H(co, dx)),
                                     rhs_h(dx, P_cur[0]), start=False,
                                     stop=(dx == 2), perf_mode=DR)

            def convout(P):
                pco = p2.tile([H, 2, 2, W], F32, tag="pco")
                for co in range(2):
                    for dx in range(3):
                        nc.tensor.matmul(pco[:, co], band(PI_O(co, dx)),
                                         rhs_h(dx, P),
                                         start=(dx == 0), stop=(dx == 2),
                                         perf_mode=DR)
                return pco

            def logprob_exp(pco):
                E = wpool.tile([H, 2, W], BF16, tag="E")
                nc.scalar.activation(E[:], pco[:, 1], AF.Exp,
                                     scale=-1.0 / S_OUT, bias=ebias[:])
                return E

            def logprob_rest(t, P, pco, E, tail=False):
                # in-loop: z path on the idle Pool engine; in the epilogue
                # (nothing to overlap) DVE is faster per op
                ztt = nc.vector.tensor_tensor if tail else \
                    nc.gpsimd.tensor_tensor
                d = wpool.tile([H, 2, W], BF16, tag="d")
                nc.vector.tensor_tensor(d[:], pco[:, 0],
                                        xs[:, t, 2 * P:2 * P + 2],
                                        op=ALU.subtract)
                z = wpool.tile([H, 2, W], BF16, tag="z")
                ztt(z[:], d[:], E[:], op=ALU.mult)
                zz = wpool.tile([H, 2, W], BF16, tag="zz")
                ztt(zz[:], z[:], z[:], op=ALU.mult)
                ztt(accz[:, 2 * P:2 * P + 2],
                    accz[:, 2 * P:2 * P + 2], zz[:], op=ALU.add)
                nc.vector.tensor_tensor(accls[:, 2 * P:2 * P + 2],
                                        accls[:, 2 * P:2 * P + 2], pco[:, 1],
                                        op=ALU.add)

            s0 = float(np.exp(-2.0 * b1) / (S_OUT * S_OUT))
            nc.vector.scalar_tensor_tensor(
                accz[:], xs[:, 0], s0, xs[:, 0], ALU.mult, ALU.mult)

            F0p5 = 0.5 / S_G
            P_cur = [0]

            for t in range(nsteps):
                for P in range(2):
                    P_cur[0] = P
                    # --- h-independent x-tap matmuls first ---
                    pig = p1.tile([H, 4, 2, W], F32, tag="pig")
                    for k, co in enumerate(IG_CO):
                        x_mms(pig[:, k], co, t, last_stop=(t == 0))
                    pfo = p2.tile([H, 4, 2, W], F32, tag="pfo")
                    for k, co in enumerate(FO_CO):
                        x_mms(pfo[:, k], co, t, last_stop=(t == 0))
                    # --- h-dependent matmuls; convout last so the gate
                    # tiles complete (and their tanhs start) sooner ---
                    if t > 0:
                        for k, co in enumerate(IG_CO):
                            h_mms(pig[:, k], co)
                        for k, co in enumerate(FO_CO):
                            h_mms(pfo[:, k], co)
                    pco = convout(P) if t > 0 else None

                    tig = wpool.tile([H, 4, 2, W], BF16, tag="tig")
                    nc.scalar.activation(tig[:], pig[:], AF.Tanh, scale=F0p5)
                    si = wpool.tile([H, 2, 2, W], BF16, tag="si")
                    nc.vector.tensor_scalar(si[:], tig[:, 0:2], 0.5, 0.5,
                                            ALU.mult, ALU.add)
                    u2 = wpool.tile([H, 2, 2, W], BF16, tag="u2")
                    nc.vector.tensor_tensor(u2[:], si[:], tig[:, 2:4],
                                            op=ALU.mult)

                    tfo = wpool.tile([H, 4, 2, W], BF16, tag="tfo")
                    nc.scalar.activation(tfo[:], pfo[:], AF.Tanh, scale=F0p5)
                    sfo = wpool.tile([H, 4, 2, W], BF16, tag="sfo")
                    nc.vector.tensor_scalar(sfo[:], tfo[:], 0.5, 0.5,
                                            ALU.mult, ALU.add)
                    cP = cst[:, :, 2 * P:2 * P + 2]
                    u1 = wpool.tile([H, 2, 2, W], BF16, tag="u1")
                    nc.vector.tensor_tensor(u1[:], sfo[:, 0:2], cP,
                                            op=ALU.mult)
                    nc.vector.tensor_tensor(cP, u1[:], u2[:], op=ALU.add)
                    # exp slots into the act gap while DVE finishes cP
                    E = logprob_exp(pco) if t > 0 else None
                    tc_ = wpool.tile([H, 2, 2, W], BF16, tag="tc")
                    nc.scalar.activation(tc_[:], cP, AF.Tanh)
                    for ci in range(2):
                        nc.vector.tensor_tensor(
                            hpair[:, ci, 2 * P:2 * P + 2, 1:1 + W],
                            sfo[:, 2 + ci], tc_[:, ci], op=ALU.mult)
                    if t > 0:
                        logprob_rest(t, P, pco, E)

            if nsteps == T:
                for P in range(2):
                    pco = convout(P)
                    E = logprob_exp(pco)
                    logprob_rest(T, P, pco, E, tail=True)

            sqr = wpool.tile([H, BL, 1], F32, tag="sqr")
            lsr = wpool.tile([H, BL, 1], F32, tag="lsr")
            nc.vector.tensor_reduce(sqr[:], accz[:],
                                    axis=mybir.AxisListType.X, op=ALU.add)
            nc.vector.tensor_reduce(lsr[:], accls[:],
                                    axis=mybir.AxisListType.X, op=ALU.add)
            ls2 = wpool.tile([H, BL], F32, tag="ls2")
            nc.vector.tensor_scalar(ls2[:], lsr[:, :, 0], 1.0 / S_OUT, None,
                                    ALU.mult)
            comb = wpool.tile([H, BL], F32, tag="comb")
            nc.vector.scalar_tensor_tensor(comb[:], sqr[:, :, 0], -0.5,
                                           ls2[:], ALU.mult, ALU.subtract)
            fps = p2.tile([BL, 1], F32, tag="pco")
            nc.tensor.matmul(fps[:], comb[:], ones[:], start=True, stop=True)
            osb = wpool.tile([BL, 1], F32, tag="osb")
            nc.vector.tensor_copy(osb[:], fps[:])
            nc.sync.dma_start(out_d[:], osb[:])

    nc.compile()
    return nc


def _get_program(b_in, b_ih, b_hh, b_out):
    key = (tuple(np.asarray(b_in, np.float32).tolist()),
           tuple(np.asarray(b_ih, np.float32).tolist()),
           tuple(np.asarray(b_hh, np.float32).tolist()),
           tuple(np.asarray(b_out, np.float32).tolist()),
           os.environ.get("KERNEL_T"))
    if key not in _CACHED:
        _CACHED[key] = _build_program(b_in, b_ih, b_hh, b_out)
    return _CACHED[key]


def kernel(x, Win, b_in, Wih, b_ih, Whh, b_hh, Wout, b_out):
    x = np.asarray(x, np.float32)
    Win = np.asarray(Win, np.float32)
    Wih = np.asarray(Wih, np.float32)
    Whh = np.asarray(Whh, np.float32)
    Wout = np.asarray(Wout, np.float32)
    b_in = np.asarray(b_in, np.float32)
    b_ih = np.asarray(b_ih, np.float32)
    b_hh = np.asarray(b_hh, np.float32)
    b_out = np.asarray(b_out, np.float32)
    b0, b1 = float(b_out[0]), float(b_out[1])

    bp = _build_bands(Win, Wih, Whh, Wout, b_in, b_ih, b_hh)
    bands_t = np.ascontiguousarray(np.transpose(bp, (2, 0, 1, 3)))
    bands8 = bands_t.astype(E4NP).reshape(H, NPAIR * 2 * H)

    in_maps = []
    for k in range(NCORES):
        xk = x[k * BL:(k + 1) * BL]
        # ship only x channels 0..14 (channel 15 never enters a conv)
        xpad = np.zeros((C - 1, H, BL, WP5), np.float32)
        xpad[:, :, :, 2:2 + W] = np.transpose(xk[:, :C - 1], (1, 2, 0, 3))
        xq = np.ascontiguousarray(
            np.transpose(xpad, (1, 0, 2, 3))).astype(E4NP).reshape(H, -1)
        xss = np.ascontiguousarray(
            np.transpose(S_OUT * (xk - b0), (2, 1, 0, 3))).astype(
                BFNP).reshape(H, -1)
        in_maps.append({"xq": xq, "xs": xss, "bands": bands8})

    nc = _get_program(b_in, b_ih, b_hh, b_out)
    global _last_in_maps
    _last_in_maps = in_maps
    res = run_bass_kernel_spmd(nc, in_maps, core_ids=list(range(NCORES)))

    const = -0.5 * LOG2PI * (H * W * C) - H * W * b1 * C
    out = np.zeros((B,), np.float32)
    for k in range(NCORES):
        out[k * BL:(k + 1) * BL] = res.results[k]["out"].reshape(BL) + const
    return out
